# revision 32
# baseline (speedup 1.0000x reference)
"""Trainium2 Bass kernel for nn_AssociatorCurrent (v2).

Math (per token t, x[t] in R^1024):
  psi_s/l/a = x @ W_* + b_*                       (three 16-dim projections)
  prod_sl  = cx(psi_s, psi_l)                     (complex-octonion product)
  prod_la  = cx(psi_l, psi_a)
  J        = cx(prod_sl, psi_a) - cx(psi_s, prod_la)
  out[t]   = J @ Jas,  Jas[i, j*16+k] = J_expand[i,j,k] - J_expand[i,k,j]

cx(x, y) with x=(a,b), y=(c,d) (8+8 split) is factored Karatsuba-style into
42-pair product rows (f[j,j,*]=0, index 7 never appears as j/k):
  H1[q] = a[jq]*c[kq], H2[q] = b[jq]*d[kq], H3[q] = (a+b)[jq]*(c+d)[kq]
  real = f.(H1 - H2),  imag = f.(H3 - H1 - H2)
giving 126 H rows per cx product and a signed reduction matrix G [126,16].

v2 algebraic restructuring (vs v1), all precomputed on host:
  * stage-2 left operand of cx(prod_sl, psi_a) is Up16.T @ (G.T @ h_sl)
    = (G@Up16).T @ h_sl  -> one [126,126] matmul straight from h_sl;
    prod_sl itself is never materialized (same for prod_la via G@Vp16).
  * the stage-1 broadcasts Usig.T@psi and Valp.T@psi are reused in stage 2
    (they appear in both stages) instead of being recomputed.
  * final contraction J.T @ Jas is fused as
    outT = (G@Jas).T @ h_left + (-G@Jas).T @ h_right
    with outT [256, tok] written transposed to DRAM; the host untransposes
    during unsharding.  PE row count drops from 11264 to 9216 per 512-token
    tile and two PSUM round-trips disappear.

Layout: features/components on partitions, tokens on the free dim.  x is
transposed on the host per shard (data-parallel over 8 cores, 2048 tokens
each).  x and the projection weights travel in X_DT (bfloat16 halves the
dominant HBM read); the H pipeline runs in W_DT.
"""

import sys

import numpy as np

try:
    import concourse  # noqa: F401  (provided on PYTHONPATH in most setups)
except ImportError:
    for _p in ("/root/.axon_site/_ro/trn_rl_repo", "/opt/trn_rl_repo"):
        if _p not in sys.path:
            sys.path.insert(0, _p)

import concourse.bass as bass
import concourse.tile as tile
from concourse import bacc
from concourse import bass_utils, mybir
from concourse.bass import ds, ts

# ---------------- problem constants (hardcoded per contest rules) ----------
B, N, D_MODEL, D_FIELD = 4, 4096, 1024, 16
NCORES = 8
TOK = B * N                      # 16384 tokens
TPC = TOK // NCORES              # 2048 tokens per core
TT = 512                         # token tile (free dim of most ops)
NT = TPC // TT                   # 4 token tiles per core
NH = 126                         # 42 pairs x 3 Karatsuba blocks
F32 = mybir.dt.float32

# dtype of x and the projection weights (the dominant DMA): bfloat16 halves
# the input traffic; float32r is exact-ish (fp32 bits, full-rate PE mode).
X_DT = mybir.dt.bfloat16
# dtype of the H pipeline (psi_sb, XL/YR copies, h tiles, constant arena).
W_DT = mybir.dt.bfloat16

_TRIPLES = [(0, 1, 2), (0, 3, 4), (0, 5, 6), (1, 3, 5), (1, 4, 6), (2, 3, 6), (2, 4, 5)]
PAIRS = [(j, k) for j in range(7) for k in range(7) if j != k]  # 42 ordered pairs


def _f_struct() -> np.ndarray:
    f = np.zeros((8, 8, 8), dtype=np.float32)
    for i, j, k in _TRIPLES:
        f[i, j, k] = 1.0
        f[j, k, i] = 1.0
        f[k, i, j] = 1.0
        f[j, i, k] = -1.0
        f[k, j, i] = -1.0
        f[i, k, j] = -1.0
    return f


def _umap(nsrc: int, ofs: int) -> np.ndarray:
    """Left-operand broadcast: row=src component, col=H row."""
    E = np.zeros((nsrc, NH), dtype=np.float32)
    for q, (j, _k) in enumerate(PAIRS):
        E[ofs + j, q] = 1.0            # H1: a[j]
        E[ofs + 8 + j, 42 + q] = 1.0   # H2: b[j]
        E[ofs + j, 84 + q] = 1.0       # H3: (a+b)[j]
        E[ofs + 8 + j, 84 + q] = 1.0
    return E


def _vmap(nsrc: int, ofs: int) -> np.ndarray:
    """Right-operand broadcast: row=src component, col=H row."""
    E = np.zeros((nsrc, NH), dtype=np.float32)
    for q, (_j, k) in enumerate(PAIRS):
        E[ofs + k, q] = 1.0            # H1: c[k]
        E[ofs + 8 + k, 42 + q] = 1.0   # H2: d[k]
        E[ofs + k, 84 + q] = 1.0       # H3: (c+d)[k]
        E[ofs + 8 + k, 84 + q] = 1.0
    return E


def _gmat() -> np.ndarray:
    """Signed structure-constant reduction [NH, 16]: H -> cx product."""
    f = _f_struct()
    G = np.zeros((NH, 16), dtype=np.float32)
    for q, (j, k) in enumerate(PAIRS):
        for i in range(8):
            fv = f[j, k, i]
            G[q, i] += fv            # real: +H1
            G[42 + q, i] -= fv       # real: -H2
            G[q, 8 + i] -= fv        # imag: -H1
            G[42 + q, 8 + i] -= fv   # imag: -H2
            G[84 + q, 8 + i] += fv   # imag: +H3
    return G


# Column offsets in the 128-row constant arena (W_DT, two DMA chunks).
OFF_USIG, OFF_VLAM, OFF_ULAM, OFF_VALP = 0, 126, 252, 378
OFF_GU16, OFF_GV16, OFF_GJAS = 504, 630, 756
NUT = 120                        # strictly-upper-triangle (j<k) jk pairs
CW = OFF_GJAS + NUT
# host-side reconstruction indices for the antisymmetric [16,16] output
_IU, _JU = np.triu_indices(D_FIELD, k=1)


def host_constants(W_sigma, b_sigma, W_lambda, b_lambda, W_alpha, b_alpha, J_expand):
    """Pack constants: arena [128, CW] in W_DT, wall [128,384] in X_DT,
    ball [48, 4] f32."""
    import ml_dtypes  # noqa: F401

    w_np = mybir.dt.np(W_DT)
    x_np = mybir.dt.np(X_DT)
    arena = np.zeros((128, CW), dtype=np.float32)
    arena[0:48, OFF_USIG:OFF_USIG + NH] = _umap(48, 0)
    arena[0:48, OFF_VLAM:OFF_VLAM + NH] = _vmap(48, 16)
    arena[0:48, OFF_ULAM:OFF_ULAM + NH] = _umap(48, 16)
    arena[0:48, OFF_VALP:OFF_VALP + NH] = _vmap(48, 32)
    g = _gmat()
    arena[0:NH, OFF_GU16:OFF_GU16 + NH] = g @ _umap(16, 0)
    arena[0:NH, OFF_GV16:OFF_GV16 + NH] = g @ _vmap(16, 0)
    jas = (J_expand - np.transpose(J_expand, (0, 2, 1))).reshape(16, 256)
    gjas = g @ jas.astype(np.float32)
    # only the strictly-upper-triangle jk columns travel to the device;
    # the host mirrors them with a sign flip (exact by antisymmetry)
    arena[0:NH, OFF_GJAS:OFF_GJAS + NUT] = gjas[:, _IU * D_FIELD + _JU]

    wall = np.concatenate([W_sigma, W_lambda, W_alpha], axis=1).astype(np.float32)
    wallx = np.ascontiguousarray(
        wall.reshape(8, 128, 48).transpose(1, 0, 2).reshape(128, 384)
    ).astype(x_np)
    ball = np.zeros((48, 4), dtype=np.float32)
    ball[:, 0] = np.concatenate([b_sigma, b_lambda, b_alpha]).astype(np.float32)
    return {
        "carena": np.ascontiguousarray(arena).astype(w_np),
        "wallx": wallx,
        "ball": ball,
    }


def build_nc(n_tiles: int = NT, repeat: int = 1):
    """Build the single-core Bass program (same program SPMDed on 8 cores).

    repeat > 1 re-runs the whole tile loop (bench-only: amortizes host
    dispatch overhead so device time can be measured from the slope).
    """
    tpc = n_tiles * TT
    nc = bacc.Bacc("TRN2", target_bir_lowering=False, debug=False)

    # x pre-tiled on host to [tile, partition, ko, token]: every partition
    # row of a tile DMA is one contiguous 8KB line
    xT = nc.dram_tensor(
        "xT", [n_tiles * 128, 8 * TT], X_DT, kind="ExternalInput"
    ).ap()
    carena = nc.dram_tensor("carena", [128, CW], W_DT, kind="ExternalInput").ap()
    wallx = nc.dram_tensor("wallx", [128, 384], X_DT, kind="ExternalInput").ap()
    ball = nc.dram_tensor("ball", [48, 4], F32, kind="ExternalInput").ap()
    # transposed compact output [120, tpc]: only j<k columns; host mirrors
    outT = nc.dram_tensor("outT", [NUT, tpc], F32, kind="ExternalOutput").ap()

    # DRAM views
    xT_r = xT.rearrange("(t p) m -> t p m", p=128)        # [NT, 128, 4096]
    xT_r4 = xT.rearrange("(t p) (ko n) -> t p ko n", p=128, n=TT)

    def mm(psum_ap, lhsT, rhs, **kw):
        nc.tensor.matmul(psum_ap, lhsT, rhs, **kw)

    with tile.TileContext(nc) as tc:
        with (
            tc.tile_pool(name="consts", bufs=1) as cpool,
            tc.tile_pool(name="xin", bufs=3) as xpool,
            tc.tile_pool(name="work", bufs=2) as wpool,
            tc.tile_pool(name="outp", bufs=2) as opool,
            tc.tile_pool(name="psum", bufs=1, space="PSUM") as pp,
        ):
            # ---- constants: tiny ball/wall first; the arena chunks are
            # issued after x tile 0 (see loop) so the first psi matmuls
            # aren't starved behind constant traffic ----
            bl = cpool.tile([48, 4], F32, tag="ball")
            nc.sync.dma_start(bl[:], ball)
            wl = cpool.tile([128, 384], X_DT, tag="wallx")
            nc.sync.dma_start(wl[:], wallx)
            ca = cpool.tile([128, CW], W_DT, tag="carena")

            wall_sb = wl.rearrange("p (ko m) -> p ko m", m=48)
            ball_sb = bl[:, 0:1]
            u_sig_sb = ca[0:48, ds(OFF_USIG, NH)]
            v_lam_sb = ca[0:48, ds(OFF_VLAM, NH)]
            u_lam_sb = ca[0:48, ds(OFF_ULAM, NH)]
            v_alp_sb = ca[0:48, ds(OFF_VALP, NH)]
            gu16_sb = ca[0:NH, ds(OFF_GU16, NH)]
            gv16_sb = ca[0:NH, ds(OFF_GV16, NH)]
            gjas_sb = ca[0:NH, ds(OFF_GJAS, NUT)]

            # PE warmup on a memset tile: ramps the PE out of the cold
            # p-state during the constant/x DMA window, with no DMA dep.
            wrm = wpool.tile([128, 512], F32, tag="warm", bufs=1)
            nc.gpsimd.memset(wrm[:], 0.0)
            wrm_r = wrm[:].bitcast(mybir.dt.float32r)
            warm_ps = pp.tile([128, 512], F32, tag="out_ps", bufs=2, name="warm_ps")
            for w in range(4):
                mm(warm_ps[:], wrm[:, ts(w % 4, 128)].bitcast(mybir.dt.float32r),
                   wrm_r, start=True, stop=True)

            psi_tiles = []
            for t in [t for _r in range(repeat) for t in range(n_tiles)]:
                # ---- load x^T tile [128, 8, TT]; tile 0 arrives in four
                # chunks so the first psi matmuls start sooner ----
                x_sb = xpool.tile([128, 8, TT], X_DT, tag="x", bufs=4)
                # every tile arrives in k-chunks so its psi matmuls start
                # while the rest of the tile is still in flight
                nch = 8 if t == 0 else 4
                kper = 8 // nch
                for c in range(nch):
                    nc.sync.dma_start(
                        x_sb[:, c * kper:(c + 1) * kper, :],
                        xT_r4[t][:, c * kper:(c + 1) * kper, :],
                    )
                if t == 0:
                    # stage-1 maps right after x tile 0
                    nc.sync.dma_start(ca[:, 0:OFF_GU16], carena[:, 0:OFF_GU16])
                elif t == 1:
                    # stage-2/out maps slot in between x tiles
                    nc.sync.dma_start(ca[:, OFF_GU16:CW], carena[:, OFF_GU16:CW])
                x_sb = x_sb[:]

                # ---- psi = W.T @ x^T + b : [48, TT] ----
                psi_ps = pp.tile([48, TT], F32, tag="psi", bufs=2)
                for k in range(8):
                    mm(psi_ps[:], wall_sb[:, k, :], x_sb[:, k, :],
                       start=(k == 0), stop=(k == 7))
                psi_sb = wpool.tile([48, TT], W_DT, tag="psi_sb", bufs=4)
                psi_tiles.append(psi_sb)
                nc.scalar.activation(
                    psi_sb[:], psi_ps[:],
                    mybir.ActivationFunctionType.Identity,
                    bias=ball_sb, scale=1.0,
                )

            for t in [t for _r in range(repeat) for t in range(n_tiles)]:
                psi_sb = psi_tiles[t]

                # ---- stage 1: h_sl, h_la (126 H rows each) ----
                # XL_sl = Usig.T@psi is copied to SBUF (reused in stage 2 as
                # the left factor of h_right); YR_sl feeds the DVE directly.
                xl_sl_ps = pp.tile([NH, TT], F32, tag="xlyr", bufs=4, name="xl_sl")
                yr_sl_ps = pp.tile([NH, TT], F32, tag="xlyr", bufs=4, name="yr_sl")
                mm(xl_sl_ps[:], u_sig_sb, psi_sb[:], start=True, stop=True)
                mm(yr_sl_ps[:], v_lam_sb, psi_sb[:], start=True, stop=True)
                xl_sig_sb = wpool.tile([NH, TT], W_DT, tag="cache", bufs=4)
                nc.scalar.activation(
                    xl_sig_sb[:], xl_sl_ps[:], mybir.ActivationFunctionType.Copy
                )
                h_sl = wpool.tile([NH, TT], W_DT, tag="h", bufs=6)
                nc.vector.tensor_mul(h_sl[:], xl_sig_sb[:], yr_sl_ps[:])

                # YR_la = Valp.T@psi is the SBUF copy (reused in stage 2 as
                # the right factor of h_left); XL_la feeds the DVE directly.
                xl_la_ps = pp.tile([NH, TT], F32, tag="xlyr", bufs=4, name="xl_la")
                yr_la_ps = pp.tile([NH, TT], F32, tag="xlyr", bufs=4, name="yr_la")
                mm(xl_la_ps[:], u_lam_sb, psi_sb[:], start=True, stop=True)
                mm(yr_la_ps[:], v_alp_sb, psi_sb[:], start=True, stop=True)
                yr_alp_sb = wpool.tile([NH, TT], W_DT, tag="cache", bufs=4)
                nc.scalar.activation(
                    yr_alp_sb[:], yr_la_ps[:], mybir.ActivationFunctionType.Copy
                )
                h_la = wpool.tile([NH, TT], W_DT, tag="h", bufs=6)
                nc.vector.tensor_mul(h_la[:], yr_alp_sb[:], xl_la_ps[:])

                # ---- stage 2: h_left, h_right via composed maps ----
                # XL_left = (G@Up16).T @ h_sl ; YR_right = (G@Vp16).T @ h_la
                # h_left/h_right stay f32 so the big left-right cancellation
                # happens before any bf16 rounding; d = h_left - h_right is
                # the only H tensor the out matmul sees.
                # The LAST tile runs stage 2 in two 256-token halves so the
                # final out DMA starts earlier (shorter drain chain).
                nhalves = 2 if t == n_tiles - 1 else 1
                hw_ = TT // nhalves
                for hh in range(nhalves):
                    cs = ds(hh * hw_, hw_)
                    xll_ps = pp.tile([NH, hw_], F32, tag="xlyr", bufs=4, name="xll")
                    mm(xll_ps[:], gu16_sb, h_sl[:, cs], start=True, stop=True)
                    h_left = wpool.tile([NH, hw_], F32, tag="hf", bufs=3)
                    nc.vector.tensor_mul(h_left[:], yr_alp_sb[:, cs], xll_ps[:])

                    yrr_ps = pp.tile([NH, hw_], F32, tag="xlyr", bufs=4, name="yrr")
                    mm(yrr_ps[:], gv16_sb, h_la[:, cs], start=True, stop=True)
                    h_right = wpool.tile([NH, hw_], F32, tag="hf", bufs=3)
                    nc.vector.tensor_mul(h_right[:], xl_sig_sb[:, cs], yrr_ps[:])

                    h_d = wpool.tile([NH, hw_], W_DT, tag="hd", bufs=2)
                    nc.vector.tensor_sub(h_d[:], h_left[:], h_right[:])

                    # ---- fused out: outT[ut, :] = GJasUT.T @ (h_l - h_r)
                    o_ps = pp.tile([NUT, hw_], F32, tag="out_ps", bufs=2)
                    mm(o_ps[:], gjas_sb, h_d[:], start=True, stop=True)
                    o_sb = opool.tile([NUT, hw_], F32, tag="out_sb", bufs=4)
                    if (t + hh) % 2 == 0:
                        nc.scalar.activation(
                            o_sb[:], o_ps[:], mybir.ActivationFunctionType.Copy
                        )
                    else:
                        nc.vector.tensor_copy(o_sb[:], o_ps[:])
                    nc.sync.dma_start(
                        outT[:, ds(t * TT + hh * hw_, hw_)], o_sb[:]
                    )

    nc.compile()
    return nc


_NC_CACHE: dict = {}


def _get_nc(n_tiles: int = NT):
    key = (n_tiles, str(W_DT), str(X_DT))
    if key not in _NC_CACHE:
        _NC_CACHE[key] = build_nc(n_tiles)
    return _NC_CACHE[key]


def _run(x, W_sigma, b_sigma, W_lambda, b_lambda, W_alpha, b_alpha, J_expand,
         **spmd_kwargs):
    consts = host_constants(
        np.asarray(W_sigma, np.float32), np.asarray(b_sigma, np.float32),
        np.asarray(W_lambda, np.float32), np.asarray(b_lambda, np.float32),
        np.asarray(W_alpha, np.float32), np.asarray(b_alpha, np.float32),
        np.asarray(J_expand, np.float32),
    )
    xflat = np.asarray(x, np.float32).reshape(TOK, D_MODEL)
    x_np_dt = mybir.dt.np(X_DT)
    in_maps = []
    for c in range(NCORES):
        xc = xflat[c * TPC:(c + 1) * TPC]          # [tpc, 1024]
        # [t, p, ko, j] = xc[t*TT+j, ko*128+p] -> contiguous 8KB lines
        xT = np.ascontiguousarray(
            xc.reshape(NT, TT, 8, 128).transpose(0, 3, 2, 1)
        ).reshape(NT * 128, 8 * TT).astype(x_np_dt)
        in_maps.append({"xT": xT, **consts})

    nc = _get_nc()
    res = bass_utils.run_bass_kernel_spmd(
        nc, in_maps, core_ids=list(range(NCORES)), **spmd_kwargs
    )
    ut = np.concatenate(
        [np.ascontiguousarray(res.results[c]["outT"].T) for c in range(NCORES)],
        axis=0,
    )  # [TOK, 120]
    out = np.zeros((TOK, D_FIELD, D_FIELD), dtype=np.float32)
    out[:, _IU, _JU] = ut
    out[:, _JU, _IU] = -ut
    return out.reshape(B, N, D_FIELD, D_FIELD), res


def kernel(x, W_sigma, b_sigma, W_lambda, b_lambda, W_alpha, b_alpha, J_expand):
    out, _ = _run(x, W_sigma, b_sigma, W_lambda, b_lambda, W_alpha, b_alpha, J_expand)
    return out


# revision 33
# speedup vs baseline: 1.1762x; 1.1762x over previous
"""Trainium2 Bass kernel for nn_AssociatorCurrent (v2).

Math (per token t, x[t] in R^1024):
  psi_s/l/a = x @ W_* + b_*                       (three 16-dim projections)
  prod_sl  = cx(psi_s, psi_l)                     (complex-octonion product)
  prod_la  = cx(psi_l, psi_a)
  J        = cx(prod_sl, psi_a) - cx(psi_s, prod_la)
  out[t]   = J @ Jas,  Jas[i, j*16+k] = J_expand[i,j,k] - J_expand[i,k,j]

cx(x, y) with x=(a,b), y=(c,d) (8+8 split) is factored Karatsuba-style into
42-pair product rows (f[j,j,*]=0, index 7 never appears as j/k):
  H1[q] = a[jq]*c[kq], H2[q] = b[jq]*d[kq], H3[q] = (a+b)[jq]*(c+d)[kq]
  real = f.(H1 - H2),  imag = f.(H3 - H1 - H2)
giving 126 H rows per cx product and a signed reduction matrix G [126,16].

v2 algebraic restructuring (vs v1), all precomputed on host:
  * stage-2 left operand of cx(prod_sl, psi_a) is Up16.T @ (G.T @ h_sl)
    = (G@Up16).T @ h_sl  -> one [126,126] matmul straight from h_sl;
    prod_sl itself is never materialized (same for prod_la via G@Vp16).
  * the stage-1 broadcasts Usig.T@psi and Valp.T@psi are reused in stage 2
    (they appear in both stages) instead of being recomputed.
  * final contraction J.T @ Jas is fused as
    outT = (G@Jas).T @ h_left + (-G@Jas).T @ h_right
    with outT [256, tok] written transposed to DRAM; the host untransposes
    during unsharding.  PE row count drops from 11264 to 9216 per 512-token
    tile and two PSUM round-trips disappear.

Layout: features/components on partitions, tokens on the free dim.  x is
transposed on the host per shard (data-parallel over 8 cores, 2048 tokens
each).  x and the projection weights travel in X_DT (bfloat16 halves the
dominant HBM read); the H pipeline runs in W_DT.
"""

import sys

import numpy as np

try:
    import concourse  # noqa: F401  (provided on PYTHONPATH in most setups)
except ImportError:
    for _p in ("/root/.axon_site/_ro/trn_rl_repo", "/opt/trn_rl_repo"):
        if _p not in sys.path:
            sys.path.insert(0, _p)

import concourse.bass as bass
import concourse.tile as tile
from concourse import bacc
from concourse import bass_utils, mybir
from concourse.bass import ds, ts

# ---------------- problem constants (hardcoded per contest rules) ----------
B, N, D_MODEL, D_FIELD = 4, 4096, 1024, 16
NCORES = 8
TOK = B * N                      # 16384 tokens
TPC = TOK // NCORES              # 2048 tokens per core
TT = 512                         # token tile (free dim of most ops)
NT = TPC // TT                   # 4 token tiles per core
NH = 126                         # 42 pairs x 3 Karatsuba blocks
F32 = mybir.dt.float32

# dtype of x and the projection weights (the dominant DMA): bfloat16 halves
# the input traffic; float32r is exact-ish (fp32 bits, full-rate PE mode).
X_DT = mybir.dt.bfloat16
# dtype of the H pipeline (psi_sb, XL/YR copies, h tiles, constant arena).
W_DT = mybir.dt.bfloat16

_TRIPLES = [(0, 1, 2), (0, 3, 4), (0, 5, 6), (1, 3, 5), (1, 4, 6), (2, 3, 6), (2, 4, 5)]
PAIRS = [(j, k) for j in range(7) for k in range(7) if j != k]  # 42 ordered pairs


def _f_struct() -> np.ndarray:
    f = np.zeros((8, 8, 8), dtype=np.float32)
    for i, j, k in _TRIPLES:
        f[i, j, k] = 1.0
        f[j, k, i] = 1.0
        f[k, i, j] = 1.0
        f[j, i, k] = -1.0
        f[k, j, i] = -1.0
        f[i, k, j] = -1.0
    return f


def _umap(nsrc: int, ofs: int) -> np.ndarray:
    """Left-operand broadcast: row=src component, col=H row."""
    E = np.zeros((nsrc, NH), dtype=np.float32)
    for q, (j, _k) in enumerate(PAIRS):
        E[ofs + j, q] = 1.0            # H1: a[j]
        E[ofs + 8 + j, 42 + q] = 1.0   # H2: b[j]
        E[ofs + j, 84 + q] = 1.0       # H3: (a+b)[j]
        E[ofs + 8 + j, 84 + q] = 1.0
    return E


def _vmap(nsrc: int, ofs: int) -> np.ndarray:
    """Right-operand broadcast: row=src component, col=H row."""
    E = np.zeros((nsrc, NH), dtype=np.float32)
    for q, (_j, k) in enumerate(PAIRS):
        E[ofs + k, q] = 1.0            # H1: c[k]
        E[ofs + 8 + k, 42 + q] = 1.0   # H2: d[k]
        E[ofs + k, 84 + q] = 1.0       # H3: (c+d)[k]
        E[ofs + 8 + k, 84 + q] = 1.0
    return E


def _gmat() -> np.ndarray:
    """Signed structure-constant reduction [NH, 16]: H -> cx product."""
    f = _f_struct()
    G = np.zeros((NH, 16), dtype=np.float32)
    for q, (j, k) in enumerate(PAIRS):
        for i in range(8):
            fv = f[j, k, i]
            G[q, i] += fv            # real: +H1
            G[42 + q, i] -= fv       # real: -H2
            G[q, 8 + i] -= fv        # imag: -H1
            G[42 + q, 8 + i] -= fv   # imag: -H2
            G[84 + q, 8 + i] += fv   # imag: +H3
    return G


# Column offsets in the 128-row constant arena (W_DT, two DMA chunks).
OFF_USIG, OFF_VLAM, OFF_ULAM, OFF_VALP = 0, 126, 252, 378
OFF_GU16, OFF_GV16, OFF_GJAS = 504, 630, 756
NUT = 120                        # strictly-upper-triangle (j<k) jk pairs
CW = OFF_GJAS + NUT
# host-side reconstruction indices for the antisymmetric [16,16] output
_IU, _JU = np.triu_indices(D_FIELD, k=1)


def host_constants(W_sigma, b_sigma, W_lambda, b_lambda, W_alpha, b_alpha, J_expand):
    """Pack constants: arena [128, CW] in W_DT, wall [128,384] in X_DT,
    ball [48, 4] f32."""
    import ml_dtypes  # noqa: F401

    w_np = mybir.dt.np(W_DT)
    x_np = mybir.dt.np(X_DT)
    arena = np.zeros((128, CW), dtype=np.float32)
    arena[0:48, OFF_USIG:OFF_USIG + NH] = _umap(48, 0)
    arena[0:48, OFF_VLAM:OFF_VLAM + NH] = _vmap(48, 16)
    arena[0:48, OFF_ULAM:OFF_ULAM + NH] = _umap(48, 16)
    arena[0:48, OFF_VALP:OFF_VALP + NH] = _vmap(48, 32)
    g = _gmat()
    arena[0:NH, OFF_GU16:OFF_GU16 + NH] = g @ _umap(16, 0)
    arena[0:NH, OFF_GV16:OFF_GV16 + NH] = g @ _vmap(16, 0)
    jas = (J_expand - np.transpose(J_expand, (0, 2, 1))).reshape(16, 256)
    gjas = g @ jas.astype(np.float32)
    # only the strictly-upper-triangle jk columns travel to the device;
    # the host mirrors them with a sign flip (exact by antisymmetry)
    arena[0:NH, OFF_GJAS:OFF_GJAS + NUT] = gjas[:, _IU * D_FIELD + _JU]

    wall = np.concatenate([W_sigma, W_lambda, W_alpha], axis=1).astype(np.float32)
    wallx = np.ascontiguousarray(
        wall.reshape(8, 128, 48).transpose(1, 0, 2).reshape(128, 384)
    ).astype(x_np)
    ball = np.zeros((48, 4), dtype=np.float32)
    ball[:, 0] = np.concatenate([b_sigma, b_lambda, b_alpha]).astype(np.float32)
    return {
        "carena": np.ascontiguousarray(arena).astype(w_np),
        "wallx": wallx,
        "ball": ball,
    }


def build_nc(n_tiles: int = NT, repeat: int = 1):
    """Build the single-core Bass program (same program SPMDed on 8 cores).

    repeat > 1 re-runs the whole tile loop (bench-only: amortizes host
    dispatch overhead so device time can be measured from the slope).
    """
    tpc = n_tiles * TT
    nc = bacc.Bacc("TRN2", target_bir_lowering=False, debug=False)

    # x pre-tiled on host to [tile, partition, ko, token]: every partition
    # row of a tile DMA is one contiguous 8KB line
    xT = nc.dram_tensor(
        "xT", [n_tiles * 128, 8 * TT], X_DT, kind="ExternalInput"
    ).ap()
    carena = nc.dram_tensor("carena", [128, CW], W_DT, kind="ExternalInput").ap()
    wallx = nc.dram_tensor("wallx", [128, 384], X_DT, kind="ExternalInput").ap()
    ball = nc.dram_tensor("ball", [48, 4], F32, kind="ExternalInput").ap()
    # transposed compact output [120, tpc]: only j<k columns; host mirrors
    outT = nc.dram_tensor("outT", [NUT, tpc], F32, kind="ExternalOutput").ap()

    # DRAM views
    xT_r = xT.rearrange("(t p) m -> t p m", p=128)        # [NT, 128, 4096]
    xT_r4 = xT.rearrange("(t p) (ko n) -> t p ko n", p=128, n=TT)

    def mm(psum_ap, lhsT, rhs, **kw):
        nc.tensor.matmul(psum_ap, lhsT, rhs, **kw)

    with tile.TileContext(nc) as tc:
        with (
            tc.tile_pool(name="consts", bufs=1) as cpool,
            tc.tile_pool(name="xin", bufs=3) as xpool,
            tc.tile_pool(name="work", bufs=2) as wpool,
            tc.tile_pool(name="outp", bufs=2) as opool,
            tc.tile_pool(name="psum", bufs=1, space="PSUM") as pp,
        ):
            # ---- constants: tiny ball/wall first; the arena chunks are
            # issued after x tile 0 (see loop) so the first psi matmuls
            # aren't starved behind constant traffic ----
            bl = cpool.tile([48, 4], F32, tag="ball")
            nc.sync.dma_start(bl[:], ball)
            wl = cpool.tile([128, 384], X_DT, tag="wallx")
            nc.sync.dma_start(wl[:], wallx)
            ca = cpool.tile([128, CW], W_DT, tag="carena")

            wall_sb = wl.rearrange("p (ko m) -> p ko m", m=48)
            ball_sb = bl[:, 0:1]
            u_sig_sb = ca[0:48, ds(OFF_USIG, NH)]
            v_lam_sb = ca[0:48, ds(OFF_VLAM, NH)]
            u_lam_sb = ca[0:48, ds(OFF_ULAM, NH)]
            v_alp_sb = ca[0:48, ds(OFF_VALP, NH)]
            gu16_sb = ca[0:NH, ds(OFF_GU16, NH)]
            gv16_sb = ca[0:NH, ds(OFF_GV16, NH)]
            gjas_sb = ca[0:NH, ds(OFF_GJAS, NUT)]

            # PE warmup on a memset tile: ramps the PE out of the cold
            # p-state during the constant/x DMA window, with no DMA dep.
            wrm = wpool.tile([128, 512], F32, tag="warm", bufs=1)
            nc.gpsimd.memset(wrm[:], 0.0)
            wrm_r = wrm[:].bitcast(mybir.dt.float32r)
            warm_ps = pp.tile([128, 512], F32, tag="out_ps", bufs=2, name="warm_ps")
            for w in range(4):
                mm(warm_ps[:], wrm[:, ts(w % 4, 128)].bitcast(mybir.dt.float32r),
                   wrm_r, start=True, stop=True)

            psi_tiles = []
            for t in [t for _r in range(repeat) for t in range(n_tiles)]:
                # ---- load x^T tile [128, 8, TT]; tile 0 arrives in four
                # chunks so the first psi matmuls start sooner ----
                x_sb = xpool.tile([128, 8, TT], X_DT, tag="x", bufs=4)
                if t == 0:
                    # tile 0 arrives in 8 chunks so psi starts ASAP
                    for c in range(8):
                        nc.sync.dma_start(
                            x_sb[:, c:c + 1, :], xT_r4[0][:, c:c + 1, :]
                        )
                    # stage-1 maps right after x tile 0
                    nc.sync.dma_start(ca[:, 0:OFF_GU16], carena[:, 0:OFF_GU16])
                else:
                    # one DMA per tile: contiguous 8KB per partition line
                    nc.sync.dma_start(
                        x_sb[:].rearrange("p ko n -> p (ko n)"), xT_r[t]
                    )
                    if t == 1:
                        # stage-2/out maps slot in between x tiles
                        nc.sync.dma_start(ca[:, OFF_GU16:CW], carena[:, OFF_GU16:CW])
                x_sb = x_sb[:]

                # ---- psi = W.T @ x^T + b : [48, TT] ----
                psi_ps = pp.tile([48, TT], F32, tag="psi", bufs=2)
                for k in range(8):
                    mm(psi_ps[:], wall_sb[:, k, :], x_sb[:, k, :],
                       start=(k == 0), stop=(k == 7))
                psi_sb = wpool.tile([48, TT], W_DT, tag="psi_sb", bufs=4)
                psi_tiles.append(psi_sb)
                nc.scalar.activation(
                    psi_sb[:], psi_ps[:],
                    mybir.ActivationFunctionType.Identity,
                    bias=ball_sb, scale=1.0,
                )

            for t in [t for _r in range(repeat) for t in range(n_tiles)]:
                psi_sb = psi_tiles[t]

                # ---- stage 1: h_sl, h_la (126 H rows each) ----
                # XL_sl = Usig.T@psi is copied to SBUF (reused in stage 2 as
                # the left factor of h_right); YR_sl feeds the DVE directly.
                xl_sl_ps = pp.tile([NH, TT], F32, tag="xlyr", bufs=4, name="xl_sl")
                yr_sl_ps = pp.tile([NH, TT], F32, tag="xlyr", bufs=4, name="yr_sl")
                mm(xl_sl_ps[:], u_sig_sb, psi_sb[:], start=True, stop=True)
                mm(yr_sl_ps[:], v_lam_sb, psi_sb[:], start=True, stop=True)
                xl_sig_sb = wpool.tile([NH, TT], W_DT, tag="cache", bufs=4)
                nc.scalar.activation(
                    xl_sig_sb[:], xl_sl_ps[:], mybir.ActivationFunctionType.Copy
                )
                h_sl = wpool.tile([NH, TT], W_DT, tag="h", bufs=6)
                nc.vector.tensor_mul(h_sl[:], xl_sig_sb[:], yr_sl_ps[:])

                # YR_la = Valp.T@psi is the SBUF copy (reused in stage 2 as
                # the right factor of h_left); XL_la feeds the DVE directly.
                xl_la_ps = pp.tile([NH, TT], F32, tag="xlyr", bufs=4, name="xl_la")
                yr_la_ps = pp.tile([NH, TT], F32, tag="xlyr", bufs=4, name="yr_la")
                mm(xl_la_ps[:], u_lam_sb, psi_sb[:], start=True, stop=True)
                mm(yr_la_ps[:], v_alp_sb, psi_sb[:], start=True, stop=True)
                yr_alp_sb = wpool.tile([NH, TT], W_DT, tag="cache", bufs=4)
                nc.scalar.activation(
                    yr_alp_sb[:], yr_la_ps[:], mybir.ActivationFunctionType.Copy
                )
                h_la = wpool.tile([NH, TT], W_DT, tag="h", bufs=6)
                nc.vector.tensor_mul(h_la[:], yr_alp_sb[:], xl_la_ps[:])

                # ---- stage 2: h_left, h_right via composed maps ----
                # XL_left = (G@Up16).T @ h_sl ; YR_right = (G@Vp16).T @ h_la
                # h_left/h_right stay f32 so the big left-right cancellation
                # happens before any bf16 rounding; d = h_left - h_right is
                # the only H tensor the out matmul sees.
                # The LAST tile runs stage 2 in two 256-token halves so the
                # final out DMA starts earlier (shorter drain chain).
                nhalves = 2 if t == n_tiles - 1 else 1
                hw_ = TT // nhalves
                for hh in range(nhalves):
                    cs = ds(hh * hw_, hw_)
                    xll_ps = pp.tile([NH, hw_], F32, tag="xlyr", bufs=4, name="xll")
                    mm(xll_ps[:], gu16_sb, h_sl[:, cs], start=True, stop=True)
                    h_left = wpool.tile([NH, hw_], F32, tag="hf", bufs=3)
                    nc.vector.tensor_mul(h_left[:], yr_alp_sb[:, cs], xll_ps[:])

                    yrr_ps = pp.tile([NH, hw_], F32, tag="xlyr", bufs=4, name="yrr")
                    mm(yrr_ps[:], gv16_sb, h_la[:, cs], start=True, stop=True)
                    h_right = wpool.tile([NH, hw_], F32, tag="hf", bufs=3)
                    nc.vector.tensor_mul(h_right[:], xl_sig_sb[:, cs], yrr_ps[:])

                    h_d = wpool.tile([NH, hw_], W_DT, tag="hd", bufs=2)
                    nc.vector.tensor_sub(h_d[:], h_left[:], h_right[:])

                    # ---- fused out: outT[ut, :] = GJasUT.T @ (h_l - h_r)
                    o_ps = pp.tile([NUT, hw_], F32, tag="out_ps", bufs=2)
                    mm(o_ps[:], gjas_sb, h_d[:], start=True, stop=True)
                    o_sb = opool.tile([NUT, hw_], F32, tag="out_sb", bufs=4)
                    if (t + hh) % 2 == 0:
                        nc.scalar.activation(
                            o_sb[:], o_ps[:], mybir.ActivationFunctionType.Copy
                        )
                    else:
                        nc.vector.tensor_copy(o_sb[:], o_ps[:])
                    nc.sync.dma_start(
                        outT[:, ds(t * TT + hh * hw_, hw_)], o_sb[:]
                    )

    nc.compile()
    return nc


_NC_CACHE: dict = {}


def _get_nc(n_tiles: int = NT):
    key = (n_tiles, str(W_DT), str(X_DT))
    if key not in _NC_CACHE:
        _NC_CACHE[key] = build_nc(n_tiles)
    return _NC_CACHE[key]


def _run(x, W_sigma, b_sigma, W_lambda, b_lambda, W_alpha, b_alpha, J_expand,
         **spmd_kwargs):
    consts = host_constants(
        np.asarray(W_sigma, np.float32), np.asarray(b_sigma, np.float32),
        np.asarray(W_lambda, np.float32), np.asarray(b_lambda, np.float32),
        np.asarray(W_alpha, np.float32), np.asarray(b_alpha, np.float32),
        np.asarray(J_expand, np.float32),
    )
    xflat = np.asarray(x, np.float32).reshape(TOK, D_MODEL)
    x_np_dt = mybir.dt.np(X_DT)
    in_maps = []
    for c in range(NCORES):
        xc = xflat[c * TPC:(c + 1) * TPC]          # [tpc, 1024]
        # [t, p, ko, j] = xc[t*TT+j, ko*128+p] -> contiguous 8KB lines
        xT = np.ascontiguousarray(
            xc.reshape(NT, TT, 8, 128).transpose(0, 3, 2, 1)
        ).reshape(NT * 128, 8 * TT).astype(x_np_dt)
        in_maps.append({"xT": xT, **consts})

    nc = _get_nc()
    res = bass_utils.run_bass_kernel_spmd(
        nc, in_maps, core_ids=list(range(NCORES)), **spmd_kwargs
    )
    ut = np.concatenate(
        [np.ascontiguousarray(res.results[c]["outT"].T) for c in range(NCORES)],
        axis=0,
    )  # [TOK, 120]
    out = np.zeros((TOK, D_FIELD, D_FIELD), dtype=np.float32)
    out[:, _IU, _JU] = ut
    out[:, _JU, _IU] = -ut
    return out.reshape(B, N, D_FIELD, D_FIELD), res


def kernel(x, W_sigma, b_sigma, W_lambda, b_lambda, W_alpha, b_alpha, J_expand):
    out, _ = _run(x, W_sigma, b_sigma, W_lambda, b_lambda, W_alpha, b_alpha, J_expand)
    return out
